# revision 1
# baseline (speedup 1.0000x reference)
"""ACSL loss kernel for 8 TRN2 NeuronCores — sorted-group edition, v3.

Reference loss (permuted-column space):
    L = [ sum_ij wm[i,j]*sp(x[i,j]) - sum_i x[i,lab_i] ] / N,  sp = softplus.

Structure exploited (original column space, after folding the reference's
column roll): bg weight vectors are unions of CONTIGUOUS column blocks
B1=[0..336] (rare), B2=[337..797] (common), B3=[798..1203] (freq +
background col, always included), so bg rows only need sum sp over one
contiguous range: plain bg rows (89% of bg) use 406 of 1204 columns,
common rows 867. fg rows need sum_j hs*sp (hs = [x >= thr]) full-width
plus an O(N) own-label host correction.

Host sorts rows into [fg | bg_common | bg_plain], pads each group to a
128-row multiple (pad x = -20: softplus exactly 0 after the fp32 1+u
bias add; fg pads vanish under the hs mask), splits groups evenly over
8 cores, and packs each core's tiles into one flat [128, TOTFREE] bf16
buffer laid out exactly as SBUF wants, so every DMA is one contiguous
rectangle. bg rows with rare_sel (two disjoint ranges, ~0.8% of rows)
are evaluated on host in f64.

Device per supertile (rs 128-row tiles of width W):
  fg:  DMA -> Exp -> Ln(bias=1) -> VectorE STT (x>=THR)*sp accum.
  bg:  DMA -> Exp -> TS t=u+1 (4x) -> TT pair-product v=t_L*t_R within
       each row (2x) -> Ln(v) with accum: ln((1+ua)(1+ub)) = sp(a)+sp(b),
       halving the Ln pass. Odd widths keep one leftover t column copied
       into v (ln(1+u) = sp directly).
Startup hiding: the first two supertiles' DMAs plus a dummy Ln/Exp pair
(which pulls the ACT table load to the queue head) are hoisted to the
very front of the program, so table load + x streaming overlap the
framework preamble barriers. Accum strips land in one [128, S] fp32
tile -> single output DMA; host sums them + corrections.
"""

import sys

for _p in ("/opt/trn_rl_repo",):
    if _p not in sys.path:
        sys.path.insert(0, _p)

import numpy as np
from ml_dtypes import bfloat16

import concourse.bass as bass
import concourse.mybir as mybir
import concourse.tile as tile
from concourse.bass_utils import run_bass_kernel_spmd

N = 16384
C = 1204
NCORES = 8
P = 128
THR = float(np.log(0.7 / 0.3))
PAD_X = -20.0
B1 = (0, 337)
B2 = (337, 798)
B3 = (798, 1204)
W_FG = C
W_C = C - B2[0]                        # 867
W_Z = C - B3[0]                        # 406
N_EARLY = 3                            # supertiles prefetched pre-preamble

_compiled = {}


def _split_waits(nc, max_waits=1):
    """Walrus codegen rejects instructions carrying more than one sem-wait;
    hoist extras onto single-wait NoOps on the same engine."""
    for fn in nc.m.functions:
        for blk in fn.blocks:
            out = []
            for inst in blk.instructions:
                si = inst.sync_info
                waits = list(si.on_wait) if si and si.on_wait else []
                if len(waits) > max_waits:
                    head, tail = waits[:-max_waits], waits[-max_waits:]
                    for j, w in enumerate(head):
                        out.append(mybir.InstNoOp(
                            name=f"{inst.name}-sw{j}",
                            engine=inst.engine,
                            ins=[], outs=[],
                            sync_info=mybir.SyncInfo(on_wait=[w],
                                                     on_update=[]),
                        ))
                    inst.sync_info = mybir.SyncInfo(
                        on_wait=tail, on_update=list(si.on_update or []))
                out.append(inst)
            blk.instructions = out


class _FastTailTC(tile.TileContext):
    """TileContext with a cheaper kernel tail: skip the dma_reset and the
    second barrier of the stock epilogue (the leading drain already
    guarantees DMA completion)."""

    def _drain_and_barrier(self, tick_clock, wait_clock):
        from concourse.bass import compact_to_ranges
        from concourse.vector_clock import ScopedClock

        drain_inst = self.nc.sync.drain()
        wait_clock.add_sem_waits(
            drain_inst.ins, ScopedClock({None: tick_clock.global_clock}))
        self.nc.all_engine_barrier()
        popped = self.nc._tile_sem_poison_stack.pop()
        assert popped is self._sem_poison
        sems = list(self.sems.allocated().values())
        sem_nums = [s.num if hasattr(s, "num") else int(s) for s in sems]
        sem_nums += getattr(self.nc, "_extra_clear_sems", [])
        for r in compact_to_ranges(sem_nums):
            self.nc.gpsimd.sem_clear(r)
        self.nc._state.prepend_free_semaphores(sem_nums)
        for poison_set in self.nc._tile_sem_poison_stack:
            poison_set.update(sem_nums)


def _supertile_split(ntiles, first_small=False, cap=4):
    sizes = []
    rem = ntiles
    if first_small and rem > 1:
        sizes.append(1)
        rem -= 1
    while rem > 0:
        s = min(cap, rem)
        sizes.append(s)
        rem -= s
    return sizes


def _geometry(nf_tiles, nc_tiles, nz_tiles):
    plan = []
    for s in _supertile_split(nf_tiles, first_small=True, cap=2):
        plan.append(("f", s, W_FG))
    for s in _supertile_split(nc_tiles, cap=6):
        plan.append(("c", s, W_C))
    for s in _supertile_split(nz_tiles, cap=6):
        plan.append(("z", s, W_Z))
    return plan


def _build_graph(plan):
    from contextlib import ExitStack
    F = mybir.ActivationFunctionType
    A = mybir.AluOpType
    nc = bass.Bass()
    totfree = sum(rs * w for _, rs, w in plan)
    n_acc = sum(rs if g == "f" else 1 for g, rs, w in plan)
    xp_d = nc.dram_tensor("xp", [P, totfree], mybir.dt.bfloat16,
                          kind="ExternalInput")
    out_d = nc.dram_tensor("out", [P, n_acc], mybir.dt.float32,
                           kind="ExternalOutput")

    ctx = ExitStack()
    # --- early block: hoisted to the program head by the surgery below ---
    early_names = []
    # dummy Ln+Exp on a scale=0 input pulls the natural_log_exp table load
    # to the head of the Scalar queue, hidden under the preamble barriers
    warm = ctx.enter_context(
        nc.sbuf_tensor("warm", [P, 2], mybir.dt.float32))
    i1 = nc.scalar.activation(warm[:, 1:2], warm[:, 0:1], F.Ln,
                              scale=0.0, bias=1.0)
    i2 = nc.scalar.activation(warm[:, 0:1], warm[:, 1:2], F.Exp, scale=0.0)
    early_names += [i1.ins.name, i2.ins.name]
    # prefetch the first N_EARLY supertiles' x DMAs
    early_sem = ctx.enter_context(nc.semaphore("early_dma"))
    early_tiles = []
    off0 = 0
    for s in range(min(N_EARLY, len(plan))):
        g, rs, w = plan[s]
        fdim = rs * w
        xt = ctx.enter_context(
            nc.sbuf_tensor(f"xe{s}", [P, fdim], mybir.dt.bfloat16))
        d = nc.sync.dma_start(xt[:], xp_d[:, off0:off0 + fdim])
        d.then_inc(early_sem, 16)
        early_names.append(d.ins.name)
        early_tiles.append(xt)
        off0 += fdim
    nc._extra_clear_sems = [early_sem.num]

    exp_waits = {}   # exp inst name -> sem wait value
    with _FastTailTC(nc) as tc:
        with (
            tc.tile_pool(name="xin", bufs=3) as xpool,
            tc.tile_pool(name="u", bufs=3) as upool,
            tc.tile_pool(name="sp", bufs=4) as sppool,
            tc.tile_pool(name="scr", bufs=2) as scrpool,
            tc.tile_pool(name="small", bufs=1) as smpool,
        ):
            acc = smpool.tile([P, n_acc], mybir.dt.float32, tag="acc")
            from concourse.bass import _add_dep_helper
            prev_act = None

            def act(*args, **kw):
                nonlocal prev_act
                inst = nc.scalar.activation(*args, **kw)
                if prev_act is not None:
                    _add_dep_helper(inst.ins, prev_act.ins, sync=False,
                                    reason="ACT order")
                prev_act = inst
                return inst

            icol_box = [0]

            def next_col():
                i = icol_box[0]
                icol_box[0] += 1
                return acc[:, i:i + 1]

            # fg STT work is queued as column chunks and drained between
            # bg VectorE ops so the 1x-rate STTs never block the bg
            # TS/TT chains feeding the Ln accums
            fg_chunks = []

            def drain_fg(k):
                for _ in range(k):
                    if not fg_chunks:
                        return
                    xt_, spt_, sl = fg_chunks.pop(0)
                    sq = scrpool.tile([P, sl.stop - sl.start],
                                      mybir.dt.bfloat16, tag="sq")
                    nc.vector.scalar_tensor_tensor(
                        out=sq[:], in0=xt_[:, sl], scalar=THR,
                        in1=spt_[:, sl], op0=A.is_ge, op1=A.mult,
                        accum_out=next_col())

            def stage1(s):
                g, rs, w = plan[s]
                fdim = rs * w
                if s < len(early_tiles):
                    xt = early_tiles[s]
                else:
                    xt = xpool.tile([P, fdim], mybir.dt.bfloat16, tag="x")
                    nc.sync.dma_start(
                        xt[:], xp_d[:, offs[s]:offs[s] + fdim])
                ut = upool.tile([P, fdim], mybir.dt.bfloat16, tag="u")
                exp_i = act(ut[:], xt[:], F.Exp)
                if s < len(early_tiles):
                    exp_waits[exp_i.ins.name] = 16 * (s + 1)
                return xt, ut

            def stage23(s, xt, ut):
                g, rs, w = plan[s]
                fdim = rs * w
                if g == "f":
                    spt = sppool.tile([P, fdim], mybir.dt.bfloat16,
                                      tag="sp")
                    act(spt[:], ut[:], F.Ln, bias=1.0)
                    for r in range(rs):
                        fg_chunks.append((xt, spt, slice(r * w, (r + 1) * w)))
                else:
                    w2 = w // 2
                    tt = sppool.tile([P, rs, w], mybir.dt.bfloat16,
                                     tag="sp")
                    nc.vector.tensor_scalar(
                        out=tt[:], in0=ut[:], scalar1=1.0, scalar2=None,
                        op0=A.add)
                    wv = w2 + (w % 2)
                    vt = scrpool.tile([P, rs, wv], mybir.dt.bfloat16,
                                      tag="vt")
                    nc.vector.tensor_tensor(
                        out=vt[:, :, 0:w2], in0=tt[:, :, 0:w2],
                        in1=tt[:, :, w2:2 * w2], op=A.mult)
                    if w % 2:
                        nc.vector.tensor_scalar(
                            out=vt[:, :, w2:wv], in0=tt[:, :, 2 * w2:w],
                            scalar1=1.0, scalar2=None, op0=A.mult)
                    lv = scrpool.tile([P, rs, wv], mybir.dt.bfloat16,
                                      tag="lv")
                    act(lv[:], vt[:], F.Ln, accum_out=next_col())
                    drain_fg(2)

            offs = []
            o = 0
            for (g, rs, w) in plan:
                offs.append(o)
                o += rs * w

            # fg supertiles: Ln right after Exp (no VE dependency to
            # cover); bg supertiles: 1-supertile lookahead so the next
            # Exp fills ACT while VectorE builds this supertile's v
            pend = None
            for s in range(len(plan)):
                cur = stage1(s)
                if pend is not None:
                    stage23(pend[0], pend[1], pend[2])
                    pend = None
                if plan[s][0] == "f":
                    stage23(s, cur[0], cur[1])
                else:
                    pend = (s, cur[0], cur[1])
            if pend is not None:
                stage23(pend[0], pend[1], pend[2])
            drain_fg(len(fg_chunks))
            nc.sync.dma_start(out_d[:], acc[:])
    ctx.close()

    # hoist the early block to the head of the entry basic block
    blk0 = nc.m.functions[0].blocks[0]
    early = [i for i in blk0.instructions if i.name in early_names]
    rest = [i for i in blk0.instructions if i.name not in early_names]
    blk0.instructions = early + rest

    # inject the early-DMA semaphore waits before the first exps
    for fn in nc.m.functions:
        for blk in fn.blocks:
            out = []
            for inst in blk.instructions:
                val = exp_waits.get(inst.name)
                if val is not None:
                    wsem = mybir.SyncWait(
                        sync_type="semaphore", id=early_sem.num,
                        ant_name="early_dma", wait_mode="sem-ge-imm",
                        wait_value=val)
                    out.append(mybir.InstNoOp(
                        name=f"{inst.name}-earlywait",
                        engine=inst.engine, ins=[], outs=[],
                        sync_info=mybir.SyncInfo(on_wait=[wsem],
                                                 on_update=[])))
                out.append(inst)
            blk.instructions = out

    _split_waits(nc)
    return nc


def _get_graph(plan):
    key = tuple(plan)
    if key not in _compiled:
        _compiled[key] = _build_graph(plan)
    return _compiled[key]


def _pack_core(xs, rows, plan, col0, totfree):
    buf = np.full((P, totfree), np.float32(PAD_X), dtype=bfloat16)
    off = 0
    pos = {g: 0 for g in rows}
    for (g, rs, w) in plan:
        nrows = rs * P
        idx = rows[g][pos[g]:pos[g] + nrows]
        pos[g] += nrows
        block = np.full((nrows, w), np.float32(PAD_X), dtype=bfloat16)
        c0 = col0[g]
        block[:len(idx)] = xs[idx, c0:c0 + w]
        buf[:, off:off + rs * w] = block.reshape(P, rs * w)
        off += rs * w
    return buf


def _prep(cls_logits, labels, rare_sel, common_sel, rare_vec, common_vec,
          freq_vec):
    x = np.asarray(cls_logits, np.float32)
    labels = np.asarray(labels).astype(np.int64)
    rare_sel = np.asarray(rare_sel).astype(bool)
    common_sel = np.asarray(common_sel).astype(bool)

    n = x.shape[0]
    is_bg = labels == C - 1
    fg = ~is_bg

    g = x[np.arange(n), labels].astype(np.float64)
    host_const = -np.sum(g)
    g_hs = (g >= THR)
    host_const += float(np.sum((np.logaddexp(0.0, g) * (1.0 - g_hs))[fg]))

    bg_r = is_bg & rare_sel & ~common_sel
    bg_rc = is_bg & rare_sel & common_sel
    if bg_r.any():
        xr = x[bg_r].astype(np.float64)
        host_const += float(
            np.logaddexp(0.0, xr[:, B1[0]:B1[1]]).sum()
            + np.logaddexp(0.0, xr[:, B3[0]:B3[1]]).sum())
    if bg_rc.any():
        host_const += float(
            np.logaddexp(0.0, x[bg_rc].astype(np.float64)).sum())

    grp_idx = {
        "f": np.nonzero(fg)[0],
        "c": np.nonzero(is_bg & common_sel & ~rare_sel)[0],
        "z": np.nonzero(is_bg & ~common_sel & ~rare_sel)[0],
    }
    col0 = {"f": 0, "c": B2[0], "z": B3[0]}

    percore = {g_: np.array_split(v, NCORES) for g_, v in grp_idx.items()}
    ntiles = {g_: max((len(ch) + P - 1) // P for ch in percore[g_])
              for g_ in percore}
    plan = _geometry(ntiles["f"], ntiles["c"], ntiles["z"])
    totfree = sum(rs * w for _, rs, w in plan)

    xs = x.astype(bfloat16)
    in_maps = []
    for ci in range(NCORES):
        rows = {g_: percore[g_][ci] for g_ in percore}
        in_maps.append({"xp": _pack_core(xs, rows, plan, col0, totfree)})
    return in_maps, plan, host_const


def _reduce(results, host_const):
    total = host_const
    for res in results:
        total += float(np.asarray(res["out"], np.float64).sum())
    return np.float32(total / N)


def kernel(cls_logits, labels, rare_sel, common_sel, rare_vec, common_vec,
           freq_vec, _run_kwargs=None):
    in_maps, plan, host_const = _prep(
        cls_logits, labels, rare_sel, common_sel, rare_vec, common_vec,
        freq_vec)
    nc = _get_graph(plan)
    kw = dict(_run_kwargs or {})
    res = run_bass_kernel_spmd(nc, in_maps, core_ids=list(range(NCORES)), **kw)
    out = _reduce(res.results, host_const)
    if kw:
        _compiled["last_results"] = res
    return out

